# revision 1
# baseline (speedup 1.0000x reference)
"""GatedGraphNeuralNetwork (GGNN) on 8 Trainium2 NeuronCores.

Sharding strategy (per the hint): nodes are sharded across the 8 cores
(6250 nodes per core). Each timestep the full node-state matrix h is
reconstructed on every core via all-gather (the halo exchange — the graph
is random, so the halo is effectively all shards). Edges are partitioned
by TARGET shard so every scatter-add is purely local to the owning core.
The small per-edge-type message weights and GRU weights are replicated.

kernel() accepts FULL inputs and returns the FULL [50000, 256] output.
"""

import numpy as np

N_NODES = 50000
HIDDEN = 256
ANNOT = 32
N_EDGE_TYPES = 4
EDGES_PER_TYPE = 75000
LAYER_TIMESTEPS = [3, 3]
N_LAYERS = 2
N_CORES = 8
SHARD = N_NODES // N_CORES  # 6250


def _partition_edges(edges):
    """Bucket edges by target shard; returns padded per-shard index arrays.

    src_idx:  [8, T, E_max] int32 global source node ids (pad -> 0)
    tgt_loc:  [8, T, E_max] int32 local target slot (pad -> SHARD trash row)
    """
    edges = np.asarray(edges).astype(np.int64)
    src = edges[:, :, 0]  # [T, E]
    tgt = edges[:, :, 1]  # [T, E]
    shard_of = tgt // SHARD

    buckets_src = [[None] * N_EDGE_TYPES for _ in range(N_CORES)]
    buckets_tgt = [[None] * N_EDGE_TYPES for _ in range(N_CORES)]
    e_max = 0
    for s in range(N_CORES):
        for t in range(N_EDGE_TYPES):
            m = shard_of[t] == s
            bs = src[t][m]
            bt = tgt[t][m] - s * SHARD
            buckets_src[s][t] = bs
            buckets_tgt[s][t] = bt
            e_max = max(e_max, bs.shape[0])

    src_idx = np.zeros((N_CORES, N_EDGE_TYPES, e_max), dtype=np.int32)
    tgt_loc = np.full((N_CORES, N_EDGE_TYPES, e_max), SHARD, dtype=np.int32)
    for s in range(N_CORES):
        for t in range(N_EDGE_TYPES):
            n = buckets_src[s][t].shape[0]
            src_idx[s, t, :n] = buckets_src[s][t]
            tgt_loc[s, t, :n] = buckets_tgt[s][t]
    return src_idx, tgt_loc, e_max


def _kernel_jax(initial_node_representation, annotations, edges, W_hid, b_hid,
                W_msg, b_msg, W_ih, W_hh, b_ih, b_hh):
    import jax
    import jax.numpy as jnp

    devs = jax.devices()
    assert len(devs) >= N_CORES, f"need {N_CORES} cores, got {len(devs)}"

    src_idx, tgt_loc, _ = _partition_edges(edges)

    x = np.asarray(initial_node_representation, np.float32).reshape(
        N_CORES, SHARD, HIDDEN)
    ann = np.asarray(annotations, np.float32).reshape(N_CORES, SHARD, ANNOT)

    W_hid = np.asarray(W_hid, np.float32)
    b_hid = np.asarray(b_hid, np.float32)
    W_msg = np.asarray(W_msg, np.float32)
    b_msg = np.asarray(b_msg, np.float32)
    W_ih = np.asarray(W_ih, np.float32)
    W_hh = np.asarray(W_hh, np.float32)
    b_ih = np.asarray(b_ih, np.float32)
    b_hh = np.asarray(b_hh, np.float32)

    def step(x_s, ann_s, src_s, tgt_s, W_hid, b_hid, W_msg, b_msg,
             W_ih, W_hh, b_ih, b_hh):
        h = jnp.concatenate([x_s, ann_s], axis=1) @ W_hid.T + b_hid
        for layer in range(N_LAYERS):
            Wm = W_msg[layer]
            bm = b_msg[layer]
            Wi, Wh = W_ih[layer], W_hh[layer]
            bi, bh = b_ih[layer], b_hh[layer]
            for _ in range(LAYER_TIMESTEPS[layer]):
                h_full = jax.lax.all_gather(
                    h, axis_name='i', tiled=True)  # [N_NODES, H]
                incoming = jnp.zeros((SHARD + 1, HIDDEN), jnp.float32)
                for t in range(N_EDGE_TYPES):
                    srcs = h_full[src_s[t]]           # [E_max, H]
                    msgs = srcs @ Wm[t].T + bm[t]
                    incoming = incoming.at[tgt_s[t]].add(msgs)
                inc = incoming[:SHARD]
                gi = inc @ Wi.T + bi
                gh = h @ Wh.T + bh
                i_r, i_z, i_n = jnp.split(gi, 3, axis=-1)
                h_r, h_z, h_n = jnp.split(gh, 3, axis=-1)
                r = jax.nn.sigmoid(i_r + h_r)
                z = jax.nn.sigmoid(i_z + h_z)
                n = jnp.tanh(i_n + r * h_n)
                h = (1.0 - z) * n + z * h
        return h

    pstep = jax.pmap(
        step, axis_name='i',
        in_axes=(0, 0, 0, 0) + (None,) * 8,
        devices=devs[:N_CORES])
    out = pstep(x, ann, src_idx, tgt_loc, W_hid, b_hid, W_msg, b_msg,
                W_ih, W_hh, b_ih, b_hh)
    return np.asarray(out).reshape(N_NODES, HIDDEN).astype(np.float32)


def _kernel_numpy(initial_node_representation, annotations, edges, W_hid,
                  b_hid, W_msg, b_msg, W_ih, W_hh, b_ih, b_hh):
    x = np.asarray(initial_node_representation, np.float32)
    ann = np.asarray(annotations, np.float32)
    edges = np.asarray(edges).astype(np.int64)
    W_hid = np.asarray(W_hid, np.float32)
    W_msg = np.asarray(W_msg, np.float32)
    b_msg = np.asarray(b_msg, np.float32)
    W_ih = np.asarray(W_ih, np.float32)
    W_hh = np.asarray(W_hh, np.float32)
    b_ih = np.asarray(b_ih, np.float32)
    b_hh = np.asarray(b_hh, np.float32)

    h = np.concatenate([x, ann], axis=1) @ W_hid.T + np.asarray(b_hid)
    sources = edges[:, :, 0]
    targets = edges[:, :, 1].reshape(-1)
    # sort targets once; segment-sum via reduceat (np.add.at is too slow)
    order = np.argsort(targets, kind='stable')
    tsorted = targets[order]
    uniq, starts = np.unique(tsorted, return_index=True)

    def sigmoid(v):
        return 1.0 / (1.0 + np.exp(-v))

    for layer in range(N_LAYERS):
        for _ in range(LAYER_TIMESTEPS[layer]):
            msgs = np.empty((N_EDGE_TYPES * EDGES_PER_TYPE, HIDDEN),
                            np.float32)
            for t in range(N_EDGE_TYPES):
                msgs[t * EDGES_PER_TYPE:(t + 1) * EDGES_PER_TYPE] = (
                    h[sources[t]] @ W_msg[layer, t].T + b_msg[layer, t])
            seg = np.add.reduceat(msgs[order], starts, axis=0)
            incoming = np.zeros((N_NODES, HIDDEN), np.float32)
            incoming[uniq] = seg
            gi = incoming @ W_ih[layer].T + b_ih[layer]
            gh = h @ W_hh[layer].T + b_hh[layer]
            r = sigmoid(gi[:, :HIDDEN] + gh[:, :HIDDEN])
            z = sigmoid(gi[:, HIDDEN:2 * HIDDEN] + gh[:, HIDDEN:2 * HIDDEN])
            n = np.tanh(gi[:, 2 * HIDDEN:] + r * gh[:, 2 * HIDDEN:])
            h = (1.0 - z) * n + z * h
    return h.astype(np.float32)


def kernel(**inputs):
    try:
        return _kernel_jax(**inputs)
    except Exception as e:  # pragma: no cover - hardware fallback
        import sys
        print(f"[kernel] jax/neuron path failed ({type(e).__name__}: {e}); "
              f"falling back to numpy", file=sys.stderr)
        return _kernel_numpy(**inputs)
